# revision 5
# baseline (speedup 1.0000x reference)
"""Trainium2 Bass kernel for nn_MultiHeadAttention_76244259439086.

Multi-head attention, B=2, S=2048, D=1024, H=16 (Dh=64), fp32 I/O.

Sharding: tensor-parallel over heads. Each of the 8 cores owns 2 adjacent
heads (a contiguous 128-column slice of Wq/Wk/Wv and the matching 128-row
slice of Wo). Every core computes q/k/v projections for its head slice,
full attention for its (batch, head) pairs, and a partial output
projection; the host sums the 8 partials and adds bo.

Device-side layouts (per core):
  xt    [128, 8, 4096]  bf16   x^T: [p, o, s] = x[s, o*128+p]
  wq/wk/wv [128, 8, 128] bf16  W slice: [p, o, m] = W[o*128+p, core_col m]
  wo    [128, 1024]     bf16   Wo rows for this core's 128 dims
  bq/bk/bv [128, 1]     f32    bias slices
  out   [4096, 1024]    f32    partial output (summed on host)

Pipeline (all matmuls via lhsT.T @ rhs on the PE):
  qT/kT [128, 4096] = (W slice).T @ x    (transposed layout: head h rows h*64..)
  v     [128, 32, 130] natural [s, d] per 128-row s-block, with a ones
        column appended per head (cols 64 and 129) so the attention matmul
        also produces the softmax denominator rows.
  scores^T [k,q] psum tiles, exp on ACT (scale=1/8) -> eT bf16,
  attn accum psum[0:64]=head0 / [64:128]=head1 (col-packed tile_position),
  denominators via ones-column lhsT (psum rows 0 and 32),
  reciprocal (DVE) -> partition_broadcast (GPSIMD) -> normalize (DVE),
  out[s,o] = attnT.T @ wo, partial DMA'd out in f32.
"""

import os
import sys
from contextlib import ExitStack

sys.path.insert(0, "/opt/trn_rl_repo")

import numpy as np
import ml_dtypes

import concourse.bass as bass
import concourse.tile as tile
from concourse import bacc, mybir
from concourse.bass import ds, ts
from concourse.bass_utils import run_bass_kernel_spmd

F32 = mybir.dt.float32
BF16 = mybir.dt.bfloat16
BF16_NP = ml_dtypes.bfloat16

B = 2
D = 1024
H = 16
DH = 64
KO = D // 128  # 8 contraction sub-tiles
N_CORES = 8
HEADS_PER_CORE = H // N_CORES  # 2


def build_program(S=2048):
    """Build + compile the per-core SPMD Bass program."""
    BS = B * S
    SB = BS // 128     # s-blocks of 128 rows
    JT = BS // 512     # 512-wide column tiles of the full token range
    QT = S // 512      # q tiles per batch
    KB = S // 128      # k blocks per batch
    SCALE = 1.0 / np.sqrt(np.float32(DH))

    nc = bacc.Bacc("TRN2", target_bir_lowering=False, debug=False,
                   enable_asserts=False)

    xt_d = nc.dram_tensor("xt", (128, KO, BS), BF16, kind="ExternalInput")
    wq_d = nc.dram_tensor("wq", (128, KO, 128), BF16, kind="ExternalInput")
    wk_d = nc.dram_tensor("wk", (128, KO, 128), BF16, kind="ExternalInput")
    wv_d = nc.dram_tensor("wv", (128, KO, 128), BF16, kind="ExternalInput")
    wo_d = nc.dram_tensor("wo", (128, D), BF16, kind="ExternalInput")
    bq_d = nc.dram_tensor("bq", (128, 1), F32, kind="ExternalInput")
    bk_d = nc.dram_tensor("bk", (128, 1), F32, kind="ExternalInput")
    bv_d = nc.dram_tensor("bv", (128, 1), F32, kind="ExternalInput")
    out_d = nc.dram_tensor("out", (BS, D), F32, kind="ExternalOutput")

    Exp = mybir.ActivationFunctionType.Exp
    mult = mybir.AluOpType.mult

    with tile.TileContext(nc) as tc:
        with ExitStack() as ctx:
            const = ctx.enter_context(tc.tile_pool(name="const", bufs=1))
            work = ctx.enter_context(tc.tile_pool(name="work", bufs=3))
            epool = ctx.enter_context(tc.tile_pool(name="epool", bufs=3))
            # PSUM budget (8 banks): scores 2x2 + attn 2x1 + den 2x1
            pool_s = ctx.enter_context(tc.tile_pool(name="ps_s", bufs=2, space="PSUM"))
            pool_at0 = ctx.enter_context(tc.tile_pool(name="ps_at0", bufs=1, space="PSUM"))
            pool_at1 = ctx.enter_context(tc.tile_pool(name="ps_at1", bufs=1, space="PSUM"))
            pool_d0 = ctx.enter_context(tc.tile_pool(name="ps_d0", bufs=1, space="PSUM"))
            pool_d1 = ctx.enter_context(tc.tile_pool(name="ps_d1", bufs=1, space="PSUM"))

            # persistent SBUF tensors
            xt = const.tile([128, KO, BS], BF16, tag="xt")
            wq = const.tile([128, KO, 128], BF16, tag="wq")
            wk = const.tile([128, KO, 128], BF16, tag="wk")
            wv = const.tile([128, KO, 128], BF16, tag="wv")
            wo = const.tile([128, D], BF16, tag="wo")
            bq = const.tile([128, 1], F32, tag="bq")
            bk = const.tile([128, 1], F32, tag="bk")
            bv = const.tile([128, 1], F32, tag="bv")
            qT = const.tile([128, BS], BF16, tag="qT")
            kT = const.tile([128, BS], BF16, tag="kT")
            v = const.tile([128, SB, 130], BF16, tag="v")
            attnT = const.tile([128, BS], BF16, tag="attnT")
            ones = const.tile([33, 64], F32, tag="ones")

            # input DMAs (xt split by column block for pipelining)
            for j in range(JT):
                nc.sync.dma_start(xt[:, :, ts(j, 512)], xt_d.ap()[:, :, ts(j, 512)])
            nc.sync.dma_start(wq[:], wq_d.ap())
            nc.sync.dma_start(wk[:], wk_d.ap())
            nc.sync.dma_start(wv[:], wv_d.ap())
            nc.sync.dma_start(wo[:], wo_d.ap())
            nc.sync.dma_start(bq[:], bq_d.ap())
            nc.sync.dma_start(bk[:], bk_d.ap())
            nc.sync.dma_start(bv[:], bv_d.ap())
            nc.any.memset(ones[:], 1.0)
            nc.any.memset(v[:, :, 64:65], 1.0)
            nc.any.memset(v[:, :, 129:130], 1.0)

            # ---- V projection, natural layout [s, d] ----
            for sb in range(SB):
                ps = pool_s.tile([128, 1024], F32, tag="s")
                for o in range(KO):
                    nc.tensor.matmul(ps[:, 0:128], lhsT=xt[:, o, ts(sb, 128)],
                                     rhs=wv[:, o], start=(o == 0), stop=(o == KO - 1))
                nc.vector.tensor_copy(v[:, sb, 0:64], ps[:, 0:64])
                nc.vector.tensor_copy(v[:, sb, 65:129], ps[:, 64:128])

            # ---- Q/K projections, transposed layout [d, s] ----
            for wmat, bias, dst in ((wq, bq, qT), (wk, bk, kT)):
                for j in range(JT):
                    ps = pool_s.tile([128, 1024], F32, tag="s")
                    for o in range(KO):
                        nc.tensor.matmul(ps[:, 0:512], lhsT=wmat[:, o],
                                         rhs=xt[:, o, ts(j, 512)],
                                         start=(o == 0), stop=(o == KO - 1))
                    nc.vector.tensor_scalar_add(dst[:, ts(j, 512)], ps[:, 0:512], bias[:])

            # ---- attention (software-pipelined over k blocks) ----
            for b in range(B):
                for qt in range(QT):
                    qs = ds(b * S + qt * 512, 512)
                    ps_at0 = pool_at0.tile([128, 512], F32, tag="at0")
                    ps_at1 = pool_at1.tile([128, 512], F32, tag="at1")
                    ps_d0 = pool_d0.tile([1, 512], F32, tag="d0")
                    ps_d1 = pool_d1.tile([33, 512], F32, tag="d1")

                    def score_exp(kb):
                        ks = ds(b * S + kb * 128, 128)
                        ps_s = pool_s.tile([128, 1024], F32, tag="s")
                        # two heads row-packed (tile_position rows 0 / 64)
                        nc.tensor.matmul(ps_s[:, 0:512], lhsT=kT[0:64, ks],
                                         rhs=qT[0:64, qs], start=True, stop=True)
                        nc.tensor.matmul(ps_s[:, 512:1024], lhsT=kT[64:128, ks],
                                         rhs=qT[64:128, qs], start=True, stop=True)
                        eT = epool.tile([128, 1024], BF16, tag="eT")
                        nc.scalar.activation(eT[:], ps_s[:], Exp, scale=float(SCALE))
                        return eT

                    def attn_acc(kb, eT):
                        sbi = b * KB + kb
                        st, sp = (kb == 0), (kb == KB - 1)
                        # one accumulation group per bank; heads at their
                        # final partition offsets (tile_position cols 0 / 64)
                        nc.tensor.matmul(ps_at0[0:64, :], lhsT=v[:, sbi, 0:64],
                                         rhs=eT[:, 0:512], start=st, stop=sp)
                        nc.tensor.matmul(ps_at1[64:128, :], lhsT=v[:, sbi, 65:129],
                                         rhs=eT[:, 512:1024], start=st, stop=sp)
                        # denominators via the ones columns
                        nc.tensor.matmul(ps_d0[0:1, :], lhsT=v[:, sbi, 64:65],
                                         rhs=eT[:, 0:512], start=st, stop=sp)
                        nc.tensor.matmul(ps_d1[32:33, :], lhsT=v[:, sbi, 129:130],
                                         rhs=eT[:, 512:1024], start=st, stop=sp)

                    prev = None
                    for kb in range(KB):
                        eT = score_exp(kb)
                        if prev is not None:
                            attn_acc(kb - 1, prev)
                        prev = eT
                    attn_acc(KB - 1, prev)

                    # normalize: 1/denom, rank-1 PE broadcast across partitions
                    recip = work.tile([33, 512], F32, tag="recip")
                    nc.vector.reciprocal(recip[0:1, :], ps_d0[0:1, :])
                    nc.vector.reciprocal(recip[32:33, :], ps_d1[32:33, :])
                    ps_bc = pool_s.tile([128, 1024], F32, tag="s")
                    nc.tensor.matmul(ps_bc[0:64, 0:512], lhsT=ones[0:1, :],
                                     rhs=recip[0:1, :], start=True, stop=True,
                                     skip_group_check=True)
                    nc.tensor.matmul(ps_bc[64:128, 0:512], lhsT=ones[32:33, :],
                                     rhs=recip[32:33, :], start=True, stop=True,
                                     skip_group_check=True)
                    # DVE may read at most one PSUM operand: stage bc in SBUF
                    bc_sb = work.tile([128, 512], F32, tag="bc")
                    nc.vector.tensor_copy(bc_sb[:], ps_bc[0:128, 0:512])
                    nc.vector.tensor_tensor(attnT[0:64, qs], ps_at0[0:64, :],
                                            bc_sb[0:64, :], mult)
                    nc.vector.tensor_scalar_add(attnT[0:64, qs], attnT[0:64, qs],
                                                bv[0:64, :])
                    nc.vector.tensor_tensor(attnT[64:128, qs], ps_at1[64:128, :],
                                            bc_sb[64:128, :], mult)
                    nc.vector.tensor_scalar_add(attnT[64:128, qs], attnT[64:128, qs],
                                                bv[64:128, :])

            # ---- output projection: out[s, o] = attnT.T @ wo (partial) ----
            for sb in range(SB):
                for ot in range(D // 512):
                    ps = pool_s.tile([128, 1024], F32, tag="s")
                    nc.tensor.matmul(ps[:, 0:512], lhsT=attnT[:, ts(sb, 128)],
                                     rhs=wo[:, ts(ot, 512)], start=True, stop=True)
                    osb = work.tile([128, 512], F32, tag="osb")
                    nc.vector.tensor_copy(osb[:], ps[:, 0:512])
                    nc.sync.dma_start(out_d.ap()[ts(sb, 128), ts(ot, 512)], osb[:])

    nc.compile()
    return nc


_CACHE = {}


def _get_program(S=2048):
    if S not in _CACHE:
        _CACHE[S] = build_program(S)
    return _CACHE[S]


def prepare_in_maps(x, Wq, bq, Wk, bk, Wv, bv, Wo, bo, S=2048):
    BS = B * S
    x = np.asarray(x, dtype=np.float32).reshape(BS, D)
    # xt[p, o, s] = x[s, o*128+p]
    xt = np.ascontiguousarray(
        x.T.reshape(KO, 128, BS).transpose(1, 0, 2)).astype(BF16_NP)

    def wslice(W, c):
        # [p, o, m] = W[o*128+p, c*128+m]
        Wc = np.asarray(W, dtype=np.float32)[:, c * 128:(c + 1) * 128]
        return np.ascontiguousarray(
            Wc.reshape(KO, 128, 128).transpose(1, 0, 2)).astype(BF16_NP)

    def bslice(bvec, c):
        return np.ascontiguousarray(
            np.asarray(bvec, dtype=np.float32)[c * 128:(c + 1) * 128]
        ).reshape(128, 1)

    in_maps = []
    for c in range(N_CORES):
        woc = np.ascontiguousarray(
            np.asarray(Wo, dtype=np.float32)[c * 128:(c + 1) * 128, :]
        ).astype(BF16_NP)
        in_maps.append({
            "xt": xt,
            "wq": wslice(Wq, c), "wk": wslice(Wk, c), "wv": wslice(Wv, c),
            "wo": woc,
            "bq": bslice(bq, c), "bk": bslice(bk, c), "bv": bslice(bv, c),
        })
    return in_maps


def run(in_maps, S=2048, trace=False, **kwargs):
    nc = _get_program(S)
    return run_bass_kernel_spmd(nc, in_maps, core_ids=list(range(N_CORES)),
                                trace=trace, **kwargs)


def kernel(x, Wq, bq, Wk, bk, Wv, bv, Wo, bo):
    S = np.asarray(x).shape[1]
    in_maps = prepare_in_maps(x, Wq, bq, Wk, bk, Wv, bv, Wo, bo, S=S)
    res = run(in_maps, S=S)
    out = np.zeros((B * S, D), dtype=np.float32)
    for r in res.results:
        out += np.asarray(r["out"], dtype=np.float32)
    out += np.asarray(bo, dtype=np.float32)[None, :]
    return out.reshape(B, S, D)
